# revision 11
# baseline (speedup 1.0000x reference)
"""Trainium2 Bass kernel for nn_MetaPosterior (loss_fn).

Math
----
Reference computes, per (a, p, k) with A=1024, P=4, K=8, D1=1025:
    theta_p = meta_theta[perm], mk_p = m_ks[k, perm], g_p = grads_v[k, perm]
    S       = sum_{r=2..D1-1} g_p[r] * (theta_p[r] - mk_p[r])
    lp      = sum_{i=0,1} [ -0.5*log(2pi) + 0.5*log(g_p[i])
                            - 0.5*g_p[i]*(theta_p[i] - mk_p[i] + S)^2 ]
(the 1/g_i and outer-product factors in the source cancel exactly).

Because perm is a true permutation of [0, D1), the tail sum telescopes:
    S = T[k] - h[k, i0] - h[k, i1],   h[k, d] = g[k, d]*(theta[d] - m_ks[k, d])
    T[k] = sum_d h[k, d],             i0, i1 = perm[0], perm[1]
so only the first two entries of each permutation are needed.

Kernel
------
Sharding: the leading 'a' axis of perms is split across the 8 NeuronCores
(128 a-values each -> 4096 (a,p,k) triples -> 8192 gather indices per core).
Small tables (g, c=theta-m_ks, 0.5*log g, h) are replicated.

Per core the device:
  1. DMAs in the 8192 combined indices (k*1025 + j, int16) and a per-partition
     T[k] column,
  2. gathers 256B table rows for all 8192 indices with one SWDGE dma_gather
     (row j of the HBM table holds [g, c, lg, h] for that (k, j)),
  3. evaluates lp for the 4096 pairs on the Vector engine and reduces to a
     [128, 1] column of partial sums,
  4. DMAs the partials out.  Host sums 8*128 partials in f64, adds the
     constant terms and the prior, and negates.
"""

import numpy as np

import concourse.bacc as bacc
import concourse.mybir as mybir
from concourse.bass_utils import run_bass_kernel_spmd
from concourse.tile import TileContext

LOG2PI = float(np.log(2.0 * np.pi))
DIM, K, P, M_COND = 1024, 8, 4, 2
D1 = DIM + 1                      # 1025
N_CORES = 8
A_PER_CORE = DIM // N_CORES       # 128
TRIPLES = A_PER_CORE * P * K      # 4096 (a', p, k) triples per core
N_IDX = 2 * TRIPLES               # 8192 gather indices per core
CHUNKS = N_IDX // 128             # 64 chunks in the gathered tile
HALF = CHUNKS // 2                # slot-0 chunks 0..31, slot-1 chunks 32..63
ROW = 64                          # table row: 64 f32 = 256 B (dma_gather min)
TBL_ROWS = K * D1                 # 8200 combined (k, j) rows

_PROGS = {}  # iters -> compiled program (built once per process)


def _build_program(iters=1, bufs=2):
    f32, i16 = mybir.dt.float32, mybir.dt.int16
    alu = mybir.AluOpType
    nc = bacc.Bacc("TRN2")

    tbl = nc.dram_tensor("tbl", [TBL_ROWS, ROW], f32, kind="ExternalInput")
    idx = nc.dram_tensor("idx", [128, N_IDX // 16], i16, kind="ExternalInput")
    tkc = nc.dram_tensor("tk", [128, 1], f32, kind="ExternalInput")
    out = nc.dram_tensor("out", [128, 1], f32, kind="ExternalOutput")

    with TileContext(nc) as tc:
        with tc.tile_pool(name="pool", bufs=1) as cpool, tc.tile_pool(
            name="work", bufs=bufs
        ) as pool:
            idx_sb = cpool.tile([128, N_IDX // 16], i16)
            tk_sb = cpool.tile([128, 1], f32)
            nc.gpsimd.dma_start(idx_sb[:], idx[:])
            nc.gpsimd.dma_start(tk_sb[:], tkc[:])

            for _ in range(iters):
                gath = pool.tile([128, CHUNKS, ROW], f32, tag="gath")
                t0 = pool.tile([128, HALF], f32, tag="t0")
                t1 = pool.tile([128, HALF], f32, tag="t1")
                t2 = pool.tile([128, HALF], f32, tag="t2")
                t3 = pool.tile([128, HALF], f32, tag="t3")
                red = pool.tile([128, 1], f32, tag="red")

                # gathered field views: slot 0 in chunks [0, HALF), slot 1
                # in [HALF, 2*HALF); field f of row m at gath[:, m, f]
                def fld(slot, f):
                    lo = slot * HALF
                    return gath[:, lo : lo + HALF, f]

                nc.gpsimd.dma_gather(
                    gath[:], tbl[:], idx_sb[:], N_IDX, N_IDX, ROW,
                    single_packet=False,
                )

                v = nc.vector
                # t0 = h0 + h1;  t0 = t0 - T[k]  (= -S)
                v.tensor_tensor(t0[:], fld(0, 3), fld(1, 3), alu.add)
                v.tensor_scalar(t0[:], t0[:], tk_sb[:], None, alu.subtract)
                # t1 = (c0 + S)^2 * g0
                v.tensor_tensor(t1[:], fld(0, 1), t0[:], alu.subtract)
                v.tensor_tensor(t1[:], t1[:], t1[:], alu.mult)
                v.tensor_tensor(t1[:], t1[:], fld(0, 0), alu.mult)
                # t2 = (c1 + S)^2 * g1
                v.tensor_tensor(t2[:], fld(1, 1), t0[:], alu.subtract)
                v.tensor_tensor(t2[:], t2[:], t2[:], alu.mult)
                v.tensor_tensor(t2[:], t2[:], fld(1, 0), alu.mult)
                # t3 = lg0 + lg1 ; t1 = t1 + t2 ; t3 = (t1 * -0.5) + t3
                v.tensor_tensor(t3[:], fld(0, 2), fld(1, 2), alu.add)
                v.tensor_tensor(t1[:], t1[:], t2[:], alu.add)
                v.scalar_tensor_tensor(t3[:], t1[:], -0.5, t3[:], alu.mult, alu.add)
                v.tensor_reduce(red[:], t3[:], mybir.AxisListType.X, alu.add)

                nc.sync.dma_start(out[:], red[:])

    nc.finalize()
    return nc


def _get_program(iters=1):
    if iters not in _PROGS:
        _PROGS[iters] = _build_program(iters)
    return _PROGS[iters]


def _device_inputs(meta_theta, m_ks, grads_v, perms):
    """Host prep: tables (O(K*D1)) and per-core index shards."""
    g = np.asarray(grads_v, np.float32)
    c = (np.asarray(meta_theta, np.float32)[None, :] - np.asarray(m_ks, np.float32))
    c = c.astype(np.float32)
    h = (g * c).astype(np.float32)
    lg = (0.5 * np.log(g.astype(np.float64))).astype(np.float32)
    t_k = h.astype(np.float64).sum(axis=1).astype(np.float32)  # (K,)

    tbl = np.zeros((TBL_ROWS, ROW), np.float32)
    tbl[:, 0] = g.ravel()
    tbl[:, 1] = c.ravel()
    tbl[:, 2] = lg.ravel()
    tbl[:, 3] = h.ravel()

    tk_col = t_k[np.arange(128) % K].reshape(128, 1).astype(np.float32)

    perms01 = np.ascontiguousarray(np.asarray(perms)[:, :, :, :2])  # (A,P,K,2)
    kvec = np.tile(np.arange(K, dtype=np.int64), TRIPLES // K)      # t = (a',p,k)

    in_maps = []
    for core in range(N_CORES):
        sl = perms01[core * A_PER_CORE : (core + 1) * A_PER_CORE]
        sl = sl.reshape(TRIPLES, 2).astype(np.int64)
        comb0 = kvec * D1 + sl[:, 0]
        comb1 = kvec * D1 + sl[:, 1]
        idx_all = np.concatenate([comb0, comb1]).astype(np.int16)   # (N_IDX,)
        # dma_gather unwraps indices as (s p) over the first 16 partitions;
        # replicate across all 8 Q7 core groups.
        idx16 = idx_all.reshape(N_IDX // 16, 16).T                  # [16, 512]
        idx128 = np.ascontiguousarray(np.tile(idx16, (8, 1)))       # [128, 512]
        in_maps.append({"tbl": tbl, "idx": idx128, "tk": tk_col})
    return in_maps


def _finalize(partials, meta_theta, alpha):
    """Combine per-core partial sums with the constant and prior terms."""
    total = float(np.sum(np.asarray(partials, np.float64)))
    sum_lp = total - LOG2PI * (N_CORES * TRIPLES)
    loss_pred = sum_lp / (P * M_COND * K)
    mt = np.asarray(meta_theta, np.float64)
    a = float(alpha)
    lp_prior = -0.5 * (D1 * LOG2PI + D1 * np.log(a) + float(mt @ mt) / a)
    loss = (1.0 - 1.0 / K) * lp_prior + loss_pred
    return np.float32(-loss)


def run_device(in_maps, iters=1, **kwargs):
    nc = _get_program(iters)
    return run_bass_kernel_spmd(nc, in_maps, list(range(N_CORES)), **kwargs)


def kernel(meta_theta, m_ks, grads_v, perms, alpha):
    in_maps = _device_inputs(meta_theta, m_ks, grads_v, perms)
    res = run_device(in_maps)
    partials = np.stack([r["out"] for r in res.results])  # (8, 128, 1)
    return _finalize(partials, meta_theta, alpha)
